# revision 8
# baseline (speedup 1.0000x reference)
"""AgentAttention block on 8 Trainium2 cores — data-parallel over batch.

Per core (one batch element, x [4096, 256]):
  qT/kT (transposed) + v (natural) projections on PE (bf16),
  stage-1 scores s1 = q @ agent_bd and stage-2-transposed s2T = k @ agent_bd
  via a block-diagonal agent matrix (both land in [n, head*agent] layout),
  exp on ACT (no max-subtraction needed: |scores*scale| <~ 2),
  stage-2 softmax denominator + agent pooling fused into one accumulated
  matmul against v augmented with a ones column,
  stage-1 softmax normalized on GPSIMD, transposed via bf16 DMA-transpose,
  final mix + output projection (fp32r) back to natural layout.

All DMAs/exps/evacs are batched at chunk (512-row) or pair (256-row)
granularity to keep the Sync/ACT instruction counts low.
"""
import numpy as np
import ml_dtypes
import concourse.bass as bass
import concourse.tile as tile
from concourse import bacc, mybir
from concourse.bass_utils import run_bass_kernel_spmd
from contextlib import ExitStack

B, N, DIM = 8, 4096, 256
H, HD, A = 8, 32, 49
SCALE = float(HD) ** -0.5
NCORES = 8
CHUNKS, CW, SUBS = 8, 512, 4
BF = mybir.dt.bfloat16
F32 = mybir.dt.float32
F32R = mybir.dt.float32r
AF = mybir.ActivationFunctionType
ALU = mybir.AluOpType


def build_nc(dbg=False):
    nc = bacc.Bacc("TRN2", target_bir_lowering=False, debug=False)
    x = nc.dram_tensor("x", [N, DIM], F32, kind="ExternalInput").ap()
    wq = nc.dram_tensor("wq", [128, 2, 256], BF, kind="ExternalInput").ap()
    wk = nc.dram_tensor("wk", [128, 2, 256], BF, kind="ExternalInput").ap()
    wv = nc.dram_tensor("wv", [128, 2, 256], BF, kind="ExternalInput").ap()
    wp = nc.dram_tensor("wp", [128, 2, 256], F32R, kind="ExternalInput").ap()
    abd = nc.dram_tensor("abd", [128, 2, 196], BF, kind="ExternalInput").ap()
    bqc = nc.dram_tensor("bqc", [128, 2], F32, kind="ExternalInput").ap()
    bkc = nc.dram_tensor("bkc", [128, 2], F32, kind="ExternalInput").ap()
    bvc = nc.dram_tensor("bvc", [128, 2], F32, kind="ExternalInput").ap()
    bpr = nc.dram_tensor("bpr", [128, 256], F32, kind="ExternalInput").ap()
    out = nc.dram_tensor("out", [N, DIM], F32, kind="ExternalOutput").ap()
    if dbg:
        d_qT = nc.dram_tensor("d_qT", [128, 2, CW], BF, kind="ExternalOutput").ap()
        d_e1n = nc.dram_tensor("d_e1n", [128, 4, 512], BF, kind="ExternalOutput").ap()
        d_va = nc.dram_tensor("d_va", [128, 2, 4, 65], BF, kind="ExternalOutput").ap()
        d_e1t = nc.dram_tensor("d_e1t", [128, 4, 4, 128], BF, kind="ExternalOutput").ap()
        d_nm = nc.dram_tensor("d_nm", [128, 4, 65], F32, kind="ExternalOutput").ap()
        d_mb = nc.dram_tensor("d_mb", [128, 4, 64], BF, kind="ExternalOutput").ap()
        d_oP = nc.dram_tensor("d_oP", [128, 2, CW], F32, kind="ExternalOutput").ap()

    with tile.TileContext(nc) as tc, ExitStack() as ctx:
        const = ctx.enter_context(tc.tile_pool(name="const", bufs=1))
        pers = ctx.enter_context(tc.tile_pool(name="pers", bufs=1))
        xsp = ctx.enter_context(tc.tile_pool(name="xsp", bufs=8))
        xtp = ctx.enter_context(tc.tile_pool(name="xtp", bufs=8))
        qkp = ctx.enter_context(tc.tile_pool(name="qkp", bufs=2))
        vap = ctx.enter_context(tc.tile_pool(name="vap", bufs=2))
        ep = ctx.enter_context(tc.tile_pool(name="ep", bufs=2))
        rp = ctx.enter_context(tc.tile_pool(name="rp", bufs=2))
        opp = ctx.enter_context(tc.tile_pool(name="opp", bufs=2))
        fop = ctx.enter_context(tc.tile_pool(name="fop", bufs=2))
        ctxA = ExitStack()
        rot = ctxA.enter_context(tc.tile_pool(name="rot", bufs=3, space="PSUM"))
        s1p = ctxA.enter_context(tc.tile_pool(name="s1p", bufs=1, space="PSUM"))
        s2p = ctxA.enter_context(tc.tile_pool(name="s2p", bufs=1, space="PSUM"))
        nmp = ctxA.enter_context(tc.tile_pool(name="nmp", bufs=1, space="PSUM"))

        wq_sb = const.tile([128, 2, 256], BF, tag="wq")
        nc.sync.dma_start(wq_sb[:], wq[:])
        wk_sb = const.tile([128, 2, 256], BF, tag="wk")
        nc.sync.dma_start(wk_sb[:], wk[:])
        wv_sb = const.tile([128, 2, 256], BF, tag="wv")
        nc.sync.dma_start(wv_sb[:], wv[:])
        wp_sb = const.tile([128, 2, 256], F32R, tag="wp")
        nc.sync.dma_start(wp_sb[:], wp[:])
        abd_sb = const.tile([128, 2, 196], BF, tag="abd")
        nc.sync.dma_start(abd_sb[:], abd[:])
        bq_sb = const.tile([128, 2], F32, tag="bq")
        nc.sync.dma_start(bq_sb[:], bqc[:])
        bk_sb = const.tile([128, 2], F32, tag="bk")
        nc.sync.dma_start(bk_sb[:], bkc[:])
        bv_sb = const.tile([128, 2], F32, tag="bv")
        nc.sync.dma_start(bv_sb[:], bvc[:])
        bp_sb = const.tile([128, 256], F32, tag="bp")
        nc.sync.dma_start(bp_sb[:], bpr[:])

        # e1t_all[p, cnk, t, g, c] = E1n[n0+128t+c, 128g+p]  (j-pad on part dim)
        e1t_all = pers.tile([128, CHUNKS, 4, 4, 128], BF, tag="e1t")
        pnm = nmp.tile([128, 4, 65], F32, tag="nm")

        # ---- rolling x prefetch: cast-load + transpose, depth 3 ----
        xT_list = []

        def load_x(cnk):
            n0 = cnk * CW
            xbf = xsp.tile([128, SUBS, DIM], BF, tag="xbf")
            nc.gpsimd.dma_start(
                xbf[:], x[n0:n0 + CW, :].rearrange("(t p) c -> p t c", p=128))
            # xTc[p, t, kb, c] = x[n0+128t+c, 128kb+p]
            xTc = xtp.tile([128, SUBS, 2, 128], BF, tag="xT")
            nc.sync.dma_start(xTc[:].rearrange("p t k c -> p (t k) c"),
                              xbf[:].rearrange("p t c -> p (t c)"),
                              transpose=True)
            xT_list.append(xTc)

        for cnk in range(3):
            load_x(cnk)

        # ---- Loop A: projections, scores, exps, stage-2 pooling ----
        for cnk in range(CHUNKS):
            n0 = cnk * CW
            if cnk + 3 < CHUNKS:
                load_x(cnk + 3)
            xTc = xT_list[cnk]
            qTc = qkp.tile([128, 2, CW], BF, tag="qT")
            kTc = qkp.tile([128, 2, CW], BF, tag="kT")
            for mb in range(2):
                ms = slice(128 * mb, 128 * mb + 128)
                pq = rot.tile([128, CW], F32, tag="rot")
                nc.tensor.matmul(pq[:], wq_sb[:, 0, ms], xTc[:, :, 0, :],
                                 start=True, stop=False)
                nc.tensor.matmul(pq[:], wq_sb[:, 1, ms], xTc[:, :, 1, :],
                                 start=False, stop=True)
                nc.scalar.activation(qTc[:, mb, :], pq[:], AF.Identity,
                                     bias=bq_sb[:, mb:mb + 1])
                pk = rot.tile([128, CW], F32, tag="rot")
                nc.tensor.matmul(pk[:], wk_sb[:, 0, ms], xTc[:, :, 0, :],
                                 start=True, stop=False)
                nc.tensor.matmul(pk[:], wk_sb[:, 1, ms], xTc[:, :, 1, :],
                                 start=False, stop=True)
                nc.vector.scalar_tensor_tensor(
                    out=kTc[:, mb, :], in0=pk[:], scalar=1.0,
                    in1=bk_sb[:, mb:mb + 1].to_broadcast((128, CW)),
                    op0=ALU.mult, op1=ALU.add)
            if dbg and cnk == 0:
                nc.sync.dma_start(d_qT[:], qTc[:])

            e1n_c = ep.tile([128, SUBS, 512], BF, tag="e1n")
            e2_c = ep.tile([128, SUBS, 512], BF, tag="e2")
            for pr in range(2):  # pairs of 128-row sub-tiles
                st0 = 2 * pr
                # v for the pair, one psum bank
                pv = rot.tile([128, 2, 256], F32, tag="rot")
                for st in (0, 1):
                    t = st0 + st
                    nc.tensor.matmul(pv[:, st, :], xTc[:, t, 0, :], wv_sb[:, 0, :],
                                     start=(st == 0), stop=False,
                                     skip_group_check=True)
                    nc.tensor.matmul(pv[:, st, :], xTc[:, t, 1, :], wv_sb[:, 1, :],
                                     start=False, stop=(st == 1),
                                     skip_group_check=True)
                vat = vap.tile([128, 2, 4, 65], BF, tag="va")
                nc.vector.tensor_copy(
                    vat[:, :, :, 0:64],
                    pv[:].rearrange("p s (g d) -> p s g d", g=4))
                nc.gpsimd.memset(vat[:, :, :, 64:65], 1.0)

                ps1 = s1p.tile([128, 2, 512], F32, tag="s1")
                ps2 = s2p.tile([128, 2, 512], F32, tag="s2")
                for st in (0, 1):
                    t = st0 + st
                    ts = slice(128 * t, 128 * t + 128)
                    for kb in range(2):
                        # kb==0 carries start=True: each 2KB bank (one per st)
                        # needs exactly one start to clear stale has_written.
                        cs = slice(196 * kb, 196 * (kb + 1))
                        nc.tensor.matmul(ps1[:, st, cs], qTc[:, kb, ts],
                                         abd_sb[:, kb, :],
                                         start=(kb == 0), stop=True,
                                         skip_group_check=True)
                        nc.tensor.matmul(ps2[:, st, cs], kTc[:, kb, ts],
                                         abd_sb[:, kb, :],
                                         start=(kb == 0), stop=True,
                                         skip_group_check=True)

                e2v = e2_c[:, st0:st0 + 2, :].rearrange("p s (h j) -> p s h j", h=8)
                nc.scalar.activation(
                    e2v[:, :, :, 0:A],
                    ps1_view(ps2), AF.Exp, scale=SCALE)
                e1v = e1n_c[:, st0:st0 + 2, :].rearrange("p s (h j) -> p s h j", h=8)
                e1raw = ep.tile([128, 2, 8, A], BF, tag="e1raw")
                nc.scalar.activation(e1raw[:], ps1_view(ps1), AF.Exp, scale=SCALE)

                r1 = rp.tile([128, 2, 8], F32, tag="r1")
                nc.vector.tensor_reduce(r1[:], e1raw[:],
                                        axis=mybir.AxisListType.X, op=ALU.add)
                r1i = rp.tile([128, 2, 8], F32, tag="r1i")
                nc.vector.reciprocal(r1i[:], r1[:])
                nc.gpsimd.tensor_mul(
                    e1v[:, :, :, 0:A], e1raw[:],
                    r1i[:].rearrange("p s (h o) -> p s h o", o=1)
                        .to_broadcast((128, 2, 8, A)))
                nc.gpsimd.memset(e1v[:, :, :, A:64], 0.0)

                for st in (0, 1):
                    t = st0 + st
                    i = cnk * SUBS + t
                    for g in range(4):
                        # exactly one start=True for the nm bank (see has_written)
                        nc.tensor.matmul(
                            pnm[:, g, :],
                            e2_c[:, t, 128 * g:128 * (g + 1)],
                            vat[:, st, g, :],
                            start=(i == 0 and g == 0), stop=(i == 31),
                            skip_group_check=True)
                if dbg and cnk == 0 and pr == 0:
                    nc.sync.dma_start(d_va[:], vat[:])

            nc.sync.dma_start(
                e1t_all[:, cnk].rearrange("p t g c -> p (t g) c"),
                e1n_c[:].rearrange("p t f -> p (t f)"), transpose=True)
            if dbg and cnk == 0:
                nc.sync.dma_start(d_e1n[:], e1n_c[:])

        if dbg:
            nc.sync.dma_start(d_e1t[:], e1t_all[:, 0])
            d_nm_sb = pers.tile([128, 4, 65], F32, tag="dnm")
            nc.vector.tensor_copy(d_nm_sb[:], pnm[:])
            nc.sync.dma_start(d_nm[:], d_nm_sb[:])
        # ---- M = x_a / c2 (per agent), block layout for the final mix ----
        mblk = pers.tile([128, 4, 64], BF, tag="mblk")
        nc.vector.memset(mblk[:], 0.0)
        for g in range(4):
            c2i = rp.tile([128, 1], F32, tag="c2i")
            nc.vector.reciprocal(c2i[:], pnm[:, g, 64:65])
            nc.vector.tensor_scalar_mul(mblk[0:A, g, 0:32], pnm[0:A, g, 0:32],
                                        c2i[0:A, 0:1])
            nc.vector.tensor_scalar_mul(mblk[64:64 + A, g, 32:64],
                                        pnm[64:64 + A, g, 32:64],
                                        c2i[64:64 + A, 0:1])
        if dbg:
            nc.sync.dma_start(d_mb[:], mblk[:])
        ctxA.close()
        rotB = ctx.enter_context(tc.tile_pool(name="rotB", bufs=6, space="PSUM"))
        # ---- Loop B: out_pre = E1n @ M (transposed), final projection ----
        for cnk in range(CHUNKS):
            n0 = cnk * CW
            pgA = rotB.tile([128, CW], F32, tag="rotB")
            pgB = rotB.tile([128, CW], F32, tag="rotB")
            for g in range(4):
                pg = pgA if g < 2 else pgB
                po = (g % 2) * 64
                nc.tensor.matmul(pg[po:po + 64, :], mblk[:, g, :],
                                 e1t_all[:, cnk, :, g, :], start=True, stop=True,
                                 skip_group_check=True)
            oPc = opp.tile([128, 2, CW], F32R, tag="oP")
            nc.scalar.activation(oPc[:, 0, :], pgA[:], AF.Identity,
                                 bias=bv_sb[:, 0:1])
            nc.scalar.activation(oPc[:, 1, :], pgB[:], AF.Identity,
                                 bias=bv_sb[:, 1:2])
            if dbg and cnk == 0:
                nc.sync.dma_start(d_oP[:], oPc[:].bitcast(F32))
            fo_c = fop.tile([128, SUBS, 256], F32, tag="fo")
            for t in range(SUBS):
                r0 = t * 128
                ts = slice(r0, r0 + 128)
                pf = rotB.tile([128, 256], F32, tag="rotB")
                nc.tensor.matmul(pf[:], oPc[:, 0, ts], wp_sb[:, 0, :],
                                 start=True, stop=False)
                nc.tensor.matmul(pf[:], oPc[:, 1, ts], wp_sb[:, 1, :],
                                 start=False, stop=True)
                nc.vector.scalar_tensor_tensor(
                    out=fo_c[:, t, :], in0=pf[:], scalar=1.0, in1=bp_sb[:],
                    op0=ALU.mult, op1=ALU.add)
            nc.gpsimd.dma_start(
                out[n0:n0 + CW, :].rearrange("(t p) c -> p t c", p=128), fo_c[:])

    nc.compile()
    return nc


def ps1_view(ps):
    return ps[:, :, 0:392].rearrange("p s (h j) -> p s h j", h=8)


_NC = None


def _get_nc():
    global _NC
    if _NC is None:
        _NC = build_nc()
    return _NC


def _prep_consts(Wq, bq, Wkv, bkv, agent_p, Wproj, bproj):
    bf = ml_dtypes.bfloat16
    f32 = np.float32

    def pack(w):  # [256, 256] -> [128, kb, 256]
        return np.ascontiguousarray(w.reshape(2, 128, 256).transpose(1, 0, 2))

    wq_h = pack(Wq).astype(bf)
    wk_h = pack(Wkv[:, 0:256]).astype(bf)
    wv_h = pack(Wkv[:, 256:512]).astype(bf)
    wp_h = pack(Wproj).astype(f32)

    ag = agent_p.reshape(A, DIM)
    abd_h = np.zeros((128, 2, 196), f32)
    for kb in range(2):
        for hh in range(4):
            d0 = 128 * kb + 32 * hh
            abd_h[32 * hh:32 * hh + 32, kb, 49 * hh:49 * hh + 49] = \
                ag[:, d0:d0 + 32].T
    abd_h = abd_h.astype(bf)

    bq_c = np.ascontiguousarray(bq.reshape(2, 128).T).astype(f32)
    bk_c = np.ascontiguousarray(bkv[0:256].reshape(2, 128).T).astype(f32)
    bv_c = np.ascontiguousarray(bkv[256:512].reshape(2, 128).T).astype(f32)
    bp_r = np.ascontiguousarray(np.broadcast_to(bproj, (128, 256))).astype(f32)
    return {"wq": wq_h, "wk": wk_h, "wv": wv_h, "wp": wp_h, "abd": abd_h,
            "bqc": bq_c, "bkc": bk_c, "bvc": bv_c, "bpr": bp_r}


def kernel(**inputs):
    x = np.asarray(inputs["x"], np.float32)
    consts = _prep_consts(
        np.asarray(inputs["Wq"], np.float32),
        np.asarray(inputs["bq"], np.float32),
        np.asarray(inputs["Wkv"], np.float32),
        np.asarray(inputs["bkv"], np.float32),
        np.asarray(inputs["agent_p"], np.float32),
        np.asarray(inputs["Wproj"], np.float32),
        np.asarray(inputs["bproj"], np.float32),
    )
    in_maps = [{**consts, "x": np.ascontiguousarray(x[b])} for b in range(B)]
    nc = _get_nc()
    res = run_bass_kernel_spmd(nc, in_maps, list(range(NCORES)))
    return np.stack([res.results[b]["out"] for b in range(B)], axis=0)
